# revision 25
# baseline (speedup 1.0000x reference)
"""Attention-pooling kernel for Trainium2 (8 NeuronCores, pure data parallel).

reference:
    h = tanh(x @ W1 + b1)            # (B,T,128)
    s = (h @ W2 + b2).squeeze(-1)    # (B,T)
    a = softmax(s, axis=1)           # (B,T)   [b2 shifts all scores equally -> no-op]
    c = einsum('bt,bth->bh', a, x)   # (B,H)
    return (c, a)

Sharding: batch dim B=128 split across 8 cores (16 rows each), params replicated.
"""

import os
import sys
import numpy as np

B, T, H, KH = 128, 2048, 256, 128
NCORES = 8
RPC = B // NCORES          # 16 rows per core
GRP = int(os.environ.get('KERNEL_GRP', '8'))  # rows per softmax/pooling group
NTC = T // 128             # 16 t-chunks of 128 per row
NTG = 4                    # t-groups per row (each 4 chunks = 512 t)

_cached = {}


def _build_nc():
    import concourse.bacc as bacc
    import concourse.bass as bass
    import concourse.mybir as mybir
    import concourse.tile as tile
    from contextlib import ExitStack

    f32 = mybir.dt.float32
    MDT = f32 if os.environ.get("KERNEL_MM_DT") == "f32" else mybir.dt.float32r
    AF = mybir.ActivationFunctionType

    nc = bacc.Bacc(None, target_bir_lowering=False)

    bf16 = mybir.dt.bfloat16
    x_d = nc.declare_dram_parameter("x", [RPC, 128, NTC * H], bf16, isOutput=False)
    xt_d = nc.declare_dram_parameter("xt", [RPC, H, T], bf16, isOutput=False)
    w1_d = nc.declare_dram_parameter("w1", [H, KH], bf16, isOutput=False)
    b1_d = nc.declare_dram_parameter("b1", [KH, 1], f32, isOutput=False)
    w2_d = nc.declare_dram_parameter("w2", [KH, 2], bf16, isOutput=False)
    eye_d = nc.declare_dram_parameter("eye", [128, 128], MDT, isOutput=False)
    attn_d = nc.declare_dram_parameter("attn_out", [RPC, T], MDT, isOutput=True)
    ctx_d = nc.declare_dram_parameter("ctx_out", [RPC, H], f32, isOutput=True)
    esum_d = nc.declare_dram_parameter("esum_out", [RPC, 1], f32, isOutput=True)
    DEBUG = bool(int(os.environ.get("KERNEL_DEBUG", "0")))
    if DEBUG:
        dbg_th = nc.declare_dram_parameter("dbg_th", [128, 512], bf16, isOutput=True)
        dbg_ssb = nc.declare_dram_parameter("dbg_ssb", [128, 64], MDT, isOutput=True)
        dbg_rows = nc.declare_dram_parameter("dbg_rows", [GRP, T], f32, isOutput=True)

    with tile.TileContext(nc) as tc, ExitStack() as ctx:
        const = ctx.enter_context(tc.tile_pool(name="const", bufs=1))
        eye_sb = const.tile([128, 128], MDT, tag="eye")
        nc.sync.dma_start(out=eye_sb[:], in_=eye_d[:, :])
        w1_sb = []
        for hc in range(2):
            t_ = const.tile([128, KH], bf16, tag=f"w1_{hc}")
            nc.sync.dma_start(out=t_[:], in_=w1_d[hc * 128:(hc + 1) * 128, :])
            w1_sb.append(t_)
        b1_sb = const.tile([KH, 1], f32, tag="b1")
        nc.sync.dma_start(out=b1_sb[:], in_=b1_d[:, :])
        w2_sb = const.tile([KH, 2], bf16, tag="w2")
        nc.sync.dma_start(out=w2_sb[:], in_=w2_d[:, :])

        x_pool = ctx.enter_context(tc.tile_pool(name="x", bufs=12))
        xt_pool = ctx.enter_context(tc.tile_pool(name="xt", bufs=10))
        h_ps = ctx.enter_context(tc.tile_pool(name="h_ps", bufs=4, space="PSUM"))
        s_ps = ctx.enter_context(tc.tile_pool(name="s_ps", bufs=1, space="PSUM"))
        misc_ps = ctx.enter_context(tc.tile_pool(name="misc_ps", bufs=2, space="PSUM"))
        tanh_p = ctx.enter_context(tc.tile_pool(name="tanh", bufs=4))
        ctx_sb = const.tile([128, (RPC // 4) * H], f32, tag="ctxsb")
        small = ctx.enter_context(tc.tile_pool(name="small", bufs=2))
        rows_p = ctx.enter_context(tc.tile_pool(name="rows", bufs=2))
        dram_p = ctx.enter_context(tc.tile_pool(name="dram", bufs=2, space="DRAM"))

        # HAM warmup: ~5us of back-to-back matmuls while the first DMAs land,
        # so the PE clock-gate opens before real work starts.
        warm_ps = h_ps.tile([128, KH], f32, tag="hx")
        for _ in range(40):
            nc.tensor.matmul(out=warm_ps[:], lhsT=w1_sb[0][:], rhs=w1_sb[1][:],
                             start=True, stop=True)

        for g in range(RPC // GRP):
            x_rows = []
            sts_dram = dram_p.tile([GRP * NTC, 128], f32, tag="sdram")
            for r4 in range(GRP):
                r = g * GRP + r4
                x_sb = x_pool.tile([128, NTC * H], bf16, tag="x")
                nc.sync.dma_start(out=x_sb[:], in_=x_d[r, :, :])
                x_rows.append(x_sb)
                xts = []
                for hc in range(2):
                    xtile = xt_pool.tile([128, T], bf16, tag="xt")
                    nc.sync.dma_start(out=xtile[:], in_=xt_d[r, hc * 128:(hc + 1) * 128, :])
                    xts.append(xtile)

                s_row = s_ps.tile([128, 2 * NTC], f32, tag="s")
                for tg in range(NTG):
                    hps = h_ps.tile([128, 512], f32, tag="hx")
                    for hc in range(2):
                        nc.tensor.matmul(
                            out=hps[:],
                            lhsT=w1_sb[hc][:],
                            rhs=xts[hc][:, tg * 512:(tg + 1) * 512],
                            start=(hc == 0),
                            stop=(hc == 1),
                        )
                    th = tanh_p.tile([128, 512], bf16, tag="tanh")
                    nc.scalar.activation(out=th[:], in_=hps[:], func=AF.Tanh, bias=b1_sb[:])
                    if DEBUG and g == 0 and r4 == 0 and tg == 0:
                        nc.sync.dma_start(out=dbg_th[:, :], in_=th[:])
                    for tc4 in range(4):
                        j = tg * 4 + tc4
                        nc.tensor.matmul(
                            out=s_row[:, 2 * j:2 * j + 2],
                            lhsT=th[:, tc4 * 128:(tc4 + 1) * 128],
                            rhs=w2_sb[:],
                            start=True, stop=True,
                        )
                s_sb_r = small.tile([128, NTC], MDT, tag="ssb")
                nc.vector.tensor_copy(
                    s_sb_r[:].rearrange("p j -> p j ()"),
                    s_row[:].rearrange("p (j two) -> p j two", two=2)[:, :, 0:1],
                )
                stp_r = misc_ps.tile([NTC, 128], MDT, tag="misc")
                nc.tensor.transpose(out=stp_r[:], in_=s_sb_r[:], identity=eye_sb[:])
                sts_r = small.tile([NTC, 128], f32, tag="sts")
                nc.vector.tensor_copy(sts_r[:], stp_r[:])
                nc.sync.dma_start(out=sts_dram[r4 * NTC:(r4 + 1) * NTC, :], in_=sts_r[:])

            # ---- per-group: load assembled scores, softmax, attn out ----
            rows = rows_p.tile([GRP, T], f32, tag="rows")
            nc.sync.dma_start(
                out=rows[:].rearrange("r (tc p) -> r tc p", tc=NTC),
                in_=sts_dram[:, :].rearrange("(r tc) p -> r tc p", r=GRP),
            )
            esum = small.tile([GRP, 1], f32, tag="esum")
            rsum = small.tile([GRP, 1], f32, tag="rsum")
            e_sb = rows_p.tile([GRP, T], MDT, tag="e")
            nc.scalar.activation(out=e_sb[:], in_=rows[:], func=AF.Exp, accum_out=esum[:])
            nc.sync.dma_start(out=esum_d[g * GRP:(g + 1) * GRP, :], in_=esum[:])

            # ---- transpose unnormalized exp to [t, r] columns (pooling weights) ----
            atp = misc_ps.tile([128, GRP * NTC], MDT, tag="misc")
            for tc in range(NTC):
                nc.tensor.transpose(
                    out=atp[:, tc * GRP:(tc + 1) * GRP],
                    in_=e_sb[:, tc * 128:(tc + 1) * 128],
                    identity=eye_sb[0:GRP, 0:GRP],
                )
            # normalized attention output (off the pooling critical path)
            attn = rows_p.tile([GRP, T], MDT, tag="attn")
            nc.vector.reciprocal(rsum[:], esum[:])
            nc.vector.tensor_scalar_mul(attn[:], e_sb[:], rsum[:])
            nc.sync.dma_start(out=attn_d[g * GRP:(g + 1) * GRP, :], in_=attn[:])
            ats = small.tile([128, GRP * NTC], bf16, tag="ats")
            nc.vector.tensor_copy(ats[:], atp[:])

            # ---- pooling: 4 rows concurrently via col-group tile_position ----
            for w4 in range(GRP // 4):
                wave = g * (GRP // 4) + w4
                cps_all = h_ps.tile([128, H], f32, tag="hx")
                for tc in range(NTC):
                    for r4 in range(4):
                        rr = w4 * 4 + r4
                        nc.tensor.matmul(
                            out=cps_all[32 * r4:32 * r4 + 1, :],
                            lhsT=ats[:, tc * GRP + rr: tc * GRP + rr + 1],
                            rhs=x_rows[rr][:, tc * H:(tc + 1) * H],
                            start=(tc == 0), stop=(tc == NTC - 1),
                            tile_position=(0, 32 * r4),
                        )
                nc.vector.tensor_copy(ctx_sb[:, wave * H:(wave + 1) * H], cps_all[:])

        nc.sync.dma_start(
            out=ctx_d[:, :].rearrange("(w r4) h -> r4 w h", r4=4),
            in_=ctx_sb[:].rearrange("(r4 z) (w h) -> r4 z w h", r4=4, w=RPC // 4)[:, 0, :, :],
        )

    nc.compile()
    return nc


def _get_runner():
    if "runner" in _cached:
        return _cached["runner"]
    sys.path.insert(0, "/opt/trn_rl_repo")
    nc = _build_nc()
    _cached["runner"] = nc
    return nc


def _make_in_maps(inputs):
    import ml_dtypes
    lstm_output = np.ascontiguousarray(inputs["lstm_output"], dtype=np.float32)
    W1, b1, W2 = inputs["W1"], inputs["b1"], inputs["W2"]
    lstm_bf = lstm_output.astype(ml_dtypes.bfloat16)
    # x tiled: [core, row, p, (tc h)] where t = tc*128 + p
    x_tiled = np.ascontiguousarray(
        lstm_bf.reshape(NCORES, RPC, NTC, 128, H).transpose(0, 1, 3, 2, 4)
    ).reshape(NCORES, RPC, 128, NTC * H)
    xt_all = np.ascontiguousarray(
        lstm_bf.reshape(NCORES, RPC, T, H).transpose(0, 1, 3, 2)
    )
    w1_bf = np.ascontiguousarray(W1, dtype=np.float32).astype(ml_dtypes.bfloat16)
    w2_bf = np.concatenate([
        np.ascontiguousarray(W2, dtype=np.float32).reshape(KH, 1),
        np.zeros((KH, 1), np.float32)], axis=1).astype(ml_dtypes.bfloat16)
    eye = np.eye(128, dtype=np.float32)
    in_maps = []
    for c in range(NCORES):
        in_maps.append({
            "x": x_tiled[c],
            "xt": xt_all[c],
            "w1": w1_bf,
            "b1": np.ascontiguousarray(b1, dtype=np.float32).reshape(KH, 1),
            "w2": w2_bf,
            "eye": eye,
        })
    return in_maps


def kernel(lstm_output, W1, b1, W2, b2):
    from concourse.bass_utils import run_bass_kernel_spmd

    nc = _get_runner()
    in_maps = _make_in_maps(dict(lstm_output=lstm_output, W1=W1, b1=b1, W2=W2, b2=b2))
    res = run_bass_kernel_spmd(
        nc, in_maps, core_ids=list(range(NCORES)),
        trace=bool(int(os.environ.get("KERNEL_TRACE", "0"))),
    )
    _cached["last_result"] = res
    outs = res.results
    context = np.concatenate([outs[c]["ctx_out"] / outs[c]["esum_out"] for c in range(NCORES)], axis=0)
    attn = np.concatenate([outs[c]["attn_out"] for c in range(NCORES)], axis=0)
    return context.astype(np.float32), attn.astype(np.float32)


# revision 26
# speedup vs baseline: 1.0454x; 1.0454x over previous
"""Attention-pooling kernel for Trainium2 (8 NeuronCores, pure data parallel).

reference:
    h = tanh(x @ W1 + b1)            # (B,T,128)
    s = (h @ W2 + b2).squeeze(-1)    # (B,T)
    a = softmax(s, axis=1)           # (B,T)   [b2 shifts all scores equally -> no-op]
    c = einsum('bt,bth->bh', a, x)   # (B,H)
    return (c, a)

Sharding: batch dim B=128 split across 8 cores (16 rows each), params replicated.
"""

import os
import sys
import numpy as np

B, T, H, KH = 128, 2048, 256, 128
NCORES = 8
RPC = B // NCORES          # 16 rows per core
GRP = int(os.environ.get('KERNEL_GRP', '8'))  # rows per softmax/pooling group
NTC = T // 128             # 16 t-chunks of 128 per row
NTG = 4                    # t-groups per row (each 4 chunks = 512 t)

_cached = {}


def _build_nc():
    import concourse.bacc as bacc
    import concourse.bass as bass
    import concourse.mybir as mybir
    import concourse.tile as tile
    from contextlib import ExitStack

    f32 = mybir.dt.float32
    MDT = f32 if os.environ.get("KERNEL_MM_DT") == "f32" else mybir.dt.float32r
    AF = mybir.ActivationFunctionType

    nc = bacc.Bacc(None, target_bir_lowering=False)

    bf16 = mybir.dt.bfloat16
    x_d = nc.declare_dram_parameter("x", [RPC, 128, NTC * H], bf16, isOutput=False)
    xt_d = nc.declare_dram_parameter("xt", [RPC, H, T], bf16, isOutput=False)
    w1_d = nc.declare_dram_parameter("w1", [H, KH], bf16, isOutput=False)
    b1_d = nc.declare_dram_parameter("b1", [KH, 1], f32, isOutput=False)
    w2_d = nc.declare_dram_parameter("w2", [KH, 2], bf16, isOutput=False)
    eye_d = nc.declare_dram_parameter("eye", [128, 128], MDT, isOutput=False)
    attn_d = nc.declare_dram_parameter("attn_out", [RPC, T], MDT, isOutput=True)
    ctx_d = nc.declare_dram_parameter("ctx_out", [RPC, H], f32, isOutput=True)
    esum_d = nc.declare_dram_parameter("esum_out", [RPC, 1], f32, isOutput=True)
    DEBUG = bool(int(os.environ.get("KERNEL_DEBUG", "0")))
    if DEBUG:
        dbg_th = nc.declare_dram_parameter("dbg_th", [128, 512], bf16, isOutput=True)
        dbg_ssb = nc.declare_dram_parameter("dbg_ssb", [128, 64], MDT, isOutput=True)
        dbg_rows = nc.declare_dram_parameter("dbg_rows", [GRP, T], f32, isOutput=True)

    with tile.TileContext(nc) as tc, ExitStack() as ctx:
        const = ctx.enter_context(tc.tile_pool(name="const", bufs=1))
        eye_sb = const.tile([128, 128], MDT, tag="eye")
        nc.sync.dma_start(out=eye_sb[:], in_=eye_d[:, :])
        w1_sb = []
        for hc in range(2):
            t_ = const.tile([128, KH], bf16, tag=f"w1_{hc}")
            nc.sync.dma_start(out=t_[:], in_=w1_d[hc * 128:(hc + 1) * 128, :])
            w1_sb.append(t_)
        b1_sb = const.tile([KH, 1], f32, tag="b1")
        nc.sync.dma_start(out=b1_sb[:], in_=b1_d[:, :])
        w2_sb = const.tile([KH, 2], bf16, tag="w2")
        nc.sync.dma_start(out=w2_sb[:], in_=w2_d[:, :])

        x_pool = ctx.enter_context(tc.tile_pool(name="x", bufs=12))
        xt_pool = ctx.enter_context(tc.tile_pool(name="xt", bufs=10))
        h_ps = ctx.enter_context(tc.tile_pool(name="h_ps", bufs=4, space="PSUM"))
        s_ps = ctx.enter_context(tc.tile_pool(name="s_ps", bufs=1, space="PSUM"))
        misc_ps = ctx.enter_context(tc.tile_pool(name="misc_ps", bufs=2, space="PSUM"))
        tanh_p = ctx.enter_context(tc.tile_pool(name="tanh", bufs=4))
        ctx_sb = const.tile([128, (RPC // 4) * H], f32, tag="ctxsb")
        small = ctx.enter_context(tc.tile_pool(name="small", bufs=2))
        rows_p = ctx.enter_context(tc.tile_pool(name="rows", bufs=2))
        dram_p = ctx.enter_context(tc.tile_pool(name="dram", bufs=2, space="DRAM"))

        # HAM warmup: ~5us of back-to-back matmuls while the first DMAs land,
        # so the PE clock-gate opens before real work starts.
        warm_ps = h_ps.tile([128, KH], f32, tag="hx")
        for _ in range(40):
            nc.tensor.matmul(out=warm_ps[:], lhsT=w1_sb[0][:], rhs=w1_sb[1][:],
                             start=True, stop=True)

        for g in range(RPC // GRP):
            x_rows = []
            s_sb_g = small.tile([128, GRP * NTC], MDT, tag="ssb")
            for r4 in range(GRP):
                r = g * GRP + r4
                x_sb = x_pool.tile([128, NTC * H], bf16, tag="x")
                nc.sync.dma_start(out=x_sb[:], in_=x_d[r, :, :])
                x_rows.append(x_sb)
                xts = []
                for hc in range(2):
                    xtile = xt_pool.tile([128, T], bf16, tag="xt")
                    nc.sync.dma_start(out=xtile[:], in_=xt_d[r, hc * 128:(hc + 1) * 128, :])
                    xts.append(xtile)

                s_row = s_ps.tile([128, 2 * NTC], f32, tag="s")
                for tg in range(NTG):
                    hps = h_ps.tile([128, 512], f32, tag="hx")
                    for hc in range(2):
                        nc.tensor.matmul(
                            out=hps[:],
                            lhsT=w1_sb[hc][:],
                            rhs=xts[hc][:, tg * 512:(tg + 1) * 512],
                            start=(hc == 0),
                            stop=(hc == 1),
                        )
                    th = tanh_p.tile([128, 512], bf16, tag="tanh")
                    nc.scalar.activation(out=th[:], in_=hps[:], func=AF.Tanh, bias=b1_sb[:])
                    if DEBUG and g == 0 and r4 == 0 and tg == 0:
                        nc.sync.dma_start(out=dbg_th[:, :], in_=th[:])
                    for tc4 in range(4):
                        j = tg * 4 + tc4
                        nc.tensor.matmul(
                            out=s_row[:, 2 * j:2 * j + 2],
                            lhsT=th[:, tc4 * 128:(tc4 + 1) * 128],
                            rhs=w2_sb[:],
                            start=True, stop=True,
                        )
                nc.vector.tensor_copy(
                    s_sb_g[:, r4 * NTC:(r4 + 1) * NTC].rearrange("p j -> p j ()"),
                    s_row[:].rearrange("p (j two) -> p j two", two=2)[:, :, 0:1],
                )

            # ---- per-group: assemble [GRP, T] scores, softmax, attn out ----
            stp = misc_ps.tile([GRP * NTC, 128], MDT, tag="misc")
            nc.tensor.transpose(out=stp[:], in_=s_sb_g[:], identity=eye_sb[:])
            sts = small.tile([GRP * NTC, 128], f32, tag="sts")
            nc.vector.tensor_copy(sts[:], stp[:])
            sts_dram = dram_p.tile([GRP * NTC, 128], f32, tag="sdram")
            nc.sync.dma_start(out=sts_dram[:], in_=sts[:])
            rows = rows_p.tile([GRP, T], f32, tag="rows")
            nc.sync.dma_start(
                out=rows[:].rearrange("r (tc p) -> r tc p", tc=NTC),
                in_=sts_dram[:, :].rearrange("(r tc) p -> r tc p", r=GRP),
            )
            esum = small.tile([GRP, 1], f32, tag="esum")
            rsum = small.tile([GRP, 1], f32, tag="rsum")
            e_sb = rows_p.tile([GRP, T], MDT, tag="e")
            nc.scalar.activation(out=e_sb[:], in_=rows[:], func=AF.Exp, accum_out=esum[:])
            nc.sync.dma_start(out=esum_d[g * GRP:(g + 1) * GRP, :], in_=esum[:])

            # ---- transpose unnormalized exp to [t, r] columns (pooling weights) ----
            atp = misc_ps.tile([128, GRP * NTC], MDT, tag="misc")
            for tc in range(NTC):
                nc.tensor.transpose(
                    out=atp[:, tc * GRP:(tc + 1) * GRP],
                    in_=e_sb[:, tc * 128:(tc + 1) * 128],
                    identity=eye_sb[0:GRP, 0:GRP],
                )
            # normalized attention output (off the pooling critical path)
            attn = rows_p.tile([GRP, T], MDT, tag="attn")
            nc.vector.reciprocal(rsum[:], esum[:])
            nc.vector.tensor_scalar_mul(attn[:], e_sb[:], rsum[:])
            nc.sync.dma_start(out=attn_d[g * GRP:(g + 1) * GRP, :], in_=attn[:])
            ats = small.tile([128, GRP * NTC], bf16, tag="ats")
            nc.vector.tensor_copy(ats[:], atp[:])

            # ---- pooling: 4 rows concurrently via col-group tile_position ----
            for w4 in range(GRP // 4):
                wave = g * (GRP // 4) + w4
                cps_all = h_ps.tile([128, H], f32, tag="hx")
                for tc in range(NTC):
                    for r4 in range(4):
                        rr = w4 * 4 + r4
                        nc.tensor.matmul(
                            out=cps_all[32 * r4:32 * r4 + 1, :],
                            lhsT=ats[:, tc * GRP + rr: tc * GRP + rr + 1],
                            rhs=x_rows[rr][:, tc * H:(tc + 1) * H],
                            start=(tc == 0), stop=(tc == NTC - 1),
                            tile_position=(0, 32 * r4),
                        )
                nc.vector.tensor_copy(ctx_sb[:, wave * H:(wave + 1) * H], cps_all[:])

        nc.sync.dma_start(
            out=ctx_d[:, :].rearrange("(w r4) h -> r4 w h", r4=4),
            in_=ctx_sb[:].rearrange("(r4 z) (w h) -> r4 z w h", r4=4, w=RPC // 4)[:, 0, :, :],
        )

    nc.compile()
    return nc


def _get_runner():
    if "runner" in _cached:
        return _cached["runner"]
    sys.path.insert(0, "/opt/trn_rl_repo")
    nc = _build_nc()
    _cached["runner"] = nc
    return nc


def _make_in_maps(inputs):
    import ml_dtypes
    lstm_output = np.ascontiguousarray(inputs["lstm_output"], dtype=np.float32)
    W1, b1, W2 = inputs["W1"], inputs["b1"], inputs["W2"]
    lstm_bf = lstm_output.astype(ml_dtypes.bfloat16)
    # x tiled: [core, row, p, (tc h)] where t = tc*128 + p
    x_tiled = np.ascontiguousarray(
        lstm_bf.reshape(NCORES, RPC, NTC, 128, H).transpose(0, 1, 3, 2, 4)
    ).reshape(NCORES, RPC, 128, NTC * H)
    xt_all = np.ascontiguousarray(
        lstm_bf.reshape(NCORES, RPC, T, H).transpose(0, 1, 3, 2)
    )
    w1_bf = np.ascontiguousarray(W1, dtype=np.float32).astype(ml_dtypes.bfloat16)
    w2_bf = np.concatenate([
        np.ascontiguousarray(W2, dtype=np.float32).reshape(KH, 1),
        np.zeros((KH, 1), np.float32)], axis=1).astype(ml_dtypes.bfloat16)
    eye = np.eye(128, dtype=np.float32)
    in_maps = []
    for c in range(NCORES):
        in_maps.append({
            "x": x_tiled[c],
            "xt": xt_all[c],
            "w1": w1_bf,
            "b1": np.ascontiguousarray(b1, dtype=np.float32).reshape(KH, 1),
            "w2": w2_bf,
            "eye": eye,
        })
    return in_maps


def kernel(lstm_output, W1, b1, W2, b2):
    from concourse.bass_utils import run_bass_kernel_spmd

    nc = _get_runner()
    in_maps = _make_in_maps(dict(lstm_output=lstm_output, W1=W1, b1=b1, W2=W2, b2=b2))
    res = run_bass_kernel_spmd(
        nc, in_maps, core_ids=list(range(NCORES)),
        trace=bool(int(os.environ.get("KERNEL_TRACE", "0"))),
    )
    _cached["last_result"] = res
    outs = res.results
    context = np.concatenate([outs[c]["ctx_out"] / outs[c]["esum_out"] for c in range(NCORES)], axis=0)
    attn = np.concatenate([outs[c]["attn_out"] for c in range(NCORES)], axis=0)
    return context.astype(np.float32), attn.astype(np.float32)


# revision 28
# speedup vs baseline: 1.1116x; 1.0633x over previous
"""Attention-pooling kernel for Trainium2 (8 NeuronCores, pure data parallel).

reference:
    h = tanh(x @ W1 + b1)            # (B,T,128)
    s = (h @ W2 + b2).squeeze(-1)    # (B,T)
    a = softmax(s, axis=1)           # (B,T)   [b2 shifts all scores equally -> no-op]
    c = einsum('bt,bth->bh', a, x)   # (B,H)
    return (c, a)

Sharding: batch dim B=128 split across 8 cores (16 rows each), params replicated.
"""

import os
import sys
import numpy as np

B, T, H, KH = 128, 2048, 256, 128
NCORES = 8
RPC = B // NCORES          # 16 rows per core
GRP = int(os.environ.get('KERNEL_GRP', '8'))  # rows per softmax/pooling group
NTC = T // 128             # 16 t-chunks of 128 per row
NTG = 4                    # t-groups per row (each 4 chunks = 512 t)

_cached = {}


def _build_nc():
    import concourse.bacc as bacc
    import concourse.bass as bass
    import concourse.mybir as mybir
    import concourse.tile as tile
    from contextlib import ExitStack

    f32 = mybir.dt.float32
    MDT = f32 if os.environ.get("KERNEL_MM_DT") == "f32" else mybir.dt.float32r
    AF = mybir.ActivationFunctionType

    nc = bacc.Bacc(None, target_bir_lowering=False)

    bf16 = mybir.dt.bfloat16
    x_d = nc.declare_dram_parameter("x", [RPC, 128, NTC * H], bf16, isOutput=False)
    xt_d = nc.declare_dram_parameter("xt", [RPC, H, T], bf16, isOutput=False)
    w1_d = nc.declare_dram_parameter("w1", [H, KH], bf16, isOutput=False)
    b1_d = nc.declare_dram_parameter("b1", [KH, 1], f32, isOutput=False)
    w2_d = nc.declare_dram_parameter("w2", [KH, 2], bf16, isOutput=False)
    eye_d = nc.declare_dram_parameter("eye", [128, 128], MDT, isOutput=False)
    attn_d = nc.declare_dram_parameter("attn_out", [RPC, T], MDT, isOutput=True)
    ctx_d = nc.declare_dram_parameter("ctx_out", [RPC, H], f32, isOutput=True)
    esum_d = nc.declare_dram_parameter("esum_out", [RPC, 1], f32, isOutput=True)
    DEBUG = bool(int(os.environ.get("KERNEL_DEBUG", "0")))
    if DEBUG:
        dbg_th = nc.declare_dram_parameter("dbg_th", [128, 512], bf16, isOutput=True)
        dbg_ssb = nc.declare_dram_parameter("dbg_ssb", [128, 64], MDT, isOutput=True)
        dbg_rows = nc.declare_dram_parameter("dbg_rows", [GRP, T], f32, isOutput=True)

    with tile.TileContext(nc) as tc, ExitStack() as ctx:
        const = ctx.enter_context(tc.tile_pool(name="const", bufs=1))
        eye_sb = const.tile([128, 128], MDT, tag="eye")
        nc.sync.dma_start(out=eye_sb[:], in_=eye_d[:, :])
        w1_sb = []
        for hc in range(2):
            t_ = const.tile([128, KH], bf16, tag=f"w1_{hc}")
            nc.sync.dma_start(out=t_[:], in_=w1_d[hc * 128:(hc + 1) * 128, :])
            w1_sb.append(t_)
        b1_sb = const.tile([KH, 1], f32, tag="b1")
        nc.sync.dma_start(out=b1_sb[:], in_=b1_d[:, :])
        w2_sb = const.tile([KH, 2], bf16, tag="w2")
        nc.sync.dma_start(out=w2_sb[:], in_=w2_d[:, :])

        x_pool = ctx.enter_context(tc.tile_pool(name="x", bufs=12))
        xt_pool = ctx.enter_context(tc.tile_pool(name="xt", bufs=10))
        xt0_pool = ctx.enter_context(tc.tile_pool(name="xt0", bufs=4))
        h_ps = ctx.enter_context(tc.tile_pool(name="h_ps", bufs=4, space="PSUM"))
        s_ps = ctx.enter_context(tc.tile_pool(name="s_ps", bufs=1, space="PSUM"))
        misc_ps = ctx.enter_context(tc.tile_pool(name="misc_ps", bufs=2, space="PSUM"))
        tanh_p = ctx.enter_context(tc.tile_pool(name="tanh", bufs=4))
        ctx_sb = const.tile([128, (RPC // 4) * H], f32, tag="ctxsb")
        small = ctx.enter_context(tc.tile_pool(name="small", bufs=2))
        rows_p = ctx.enter_context(tc.tile_pool(name="rows", bufs=2))
        dram_p = ctx.enter_context(tc.tile_pool(name="dram", bufs=2, space="DRAM"))

        # HAM warmup: ~5us of back-to-back matmuls while the first DMAs land,
        # so the PE clock-gate opens before real work starts.
        warm_ps = h_ps.tile([128, KH], f32, tag="hx")
        for _ in range(40):
            nc.tensor.matmul(out=warm_ps[:], lhsT=w1_sb[0][:], rhs=w1_sb[1][:],
                             start=True, stop=True)

        for g in range(RPC // GRP):
            x_rows = []
            s_sb_g = small.tile([128, GRP * NTC], MDT, tag="ssb")
            for r4 in range(GRP):
                r = g * GRP + r4
                # xt first (gates the score matmuls); x is only needed at pooling
                if r == 0:
                    # fine-grained first-row xt so the PE starts ~4x earlier
                    xth = []
                    for hc in range(2):
                        row = []
                        for hf in range(2):
                            t4 = xt0_pool.tile([128, T // 2], bf16, tag="xt0")
                            nc.sync.dma_start(
                                out=t4[:],
                                in_=xt_d[r, hc * 128:(hc + 1) * 128,
                                         hf * (T // 2):(hf + 1) * (T // 2)])
                            row.append(t4)
                        xth.append(row)
                    xts = None
                else:
                    xts = []
                    for hc in range(2):
                        xtile = xt_pool.tile([128, T], bf16, tag="xt")
                        nc.sync.dma_start(out=xtile[:], in_=xt_d[r, hc * 128:(hc + 1) * 128, :])
                        xts.append(xtile)
                x_sb = x_pool.tile([128, NTC * H], bf16, tag="x")
                nc.sync.dma_start(out=x_sb[:], in_=x_d[r, :, :])
                x_rows.append(x_sb)

                s_row = s_ps.tile([128, 2 * NTC], f32, tag="s")
                for tg in range(NTG):
                    hps = h_ps.tile([128, 512], f32, tag="hx")
                    for hc in range(2):
                        if r == 0:
                            rhs_ap = xth[hc][tg // 2][:, (tg % 2) * 512:(tg % 2) * 512 + 512]
                        else:
                            rhs_ap = xts[hc][:, tg * 512:(tg + 1) * 512]
                        nc.tensor.matmul(
                            out=hps[:],
                            lhsT=w1_sb[hc][:],
                            rhs=rhs_ap,
                            start=(hc == 0),
                            stop=(hc == 1),
                        )
                    th = tanh_p.tile([128, 512], bf16, tag="tanh")
                    nc.scalar.activation(out=th[:], in_=hps[:], func=AF.Tanh, bias=b1_sb[:])
                    if DEBUG and g == 0 and r4 == 0 and tg == 0:
                        nc.sync.dma_start(out=dbg_th[:, :], in_=th[:])
                    for tc4 in range(4):
                        j = tg * 4 + tc4
                        nc.tensor.matmul(
                            out=s_row[:, 2 * j:2 * j + 2],
                            lhsT=th[:, tc4 * 128:(tc4 + 1) * 128],
                            rhs=w2_sb[:],
                            start=True, stop=True,
                        )
                nc.vector.tensor_copy(
                    s_sb_g[:, r4 * NTC:(r4 + 1) * NTC].rearrange("p j -> p j ()"),
                    s_row[:].rearrange("p (j two) -> p j two", two=2)[:, :, 0:1],
                )

            # ---- per-group: assemble [GRP, T] scores, softmax, attn out ----
            stp = misc_ps.tile([GRP * NTC, 128], MDT, tag="misc")
            nc.tensor.transpose(out=stp[:], in_=s_sb_g[:], identity=eye_sb[:])
            sts = small.tile([GRP * NTC, 128], f32, tag="sts")
            nc.vector.tensor_copy(sts[:], stp[:])
            sts_dram = dram_p.tile([GRP * NTC, 128], f32, tag="sdram")
            nc.sync.dma_start(out=sts_dram[:], in_=sts[:])
            rows = rows_p.tile([GRP, T], f32, tag="rows")
            nc.sync.dma_start(
                out=rows[:].rearrange("r (tc p) -> r tc p", tc=NTC),
                in_=sts_dram[:, :].rearrange("(r tc) p -> r tc p", r=GRP),
            )
            esum = small.tile([GRP, 1], f32, tag="esum")
            rsum = small.tile([GRP, 1], f32, tag="rsum")
            e_sb = rows_p.tile([GRP, T], MDT, tag="e")
            nc.scalar.activation(out=e_sb[:], in_=rows[:], func=AF.Exp, accum_out=esum[:])
            nc.sync.dma_start(out=esum_d[g * GRP:(g + 1) * GRP, :], in_=esum[:])

            # ---- transpose unnormalized exp to [t, r] columns (pooling weights) ----
            atp = misc_ps.tile([128, GRP * NTC], MDT, tag="misc")
            for tc in range(NTC):
                nc.tensor.transpose(
                    out=atp[:, tc * GRP:(tc + 1) * GRP],
                    in_=e_sb[:, tc * 128:(tc + 1) * 128],
                    identity=eye_sb[0:GRP, 0:GRP],
                )
            # normalized attention output (off the pooling critical path)
            attn = rows_p.tile([GRP, T], MDT, tag="attn")
            nc.vector.reciprocal(rsum[:], esum[:])
            nc.vector.tensor_scalar_mul(attn[:], e_sb[:], rsum[:])
            nc.sync.dma_start(out=attn_d[g * GRP:(g + 1) * GRP, :], in_=attn[:])
            ats = small.tile([128, GRP * NTC], bf16, tag="ats")
            nc.vector.tensor_copy(ats[:], atp[:])

            # ---- pooling: 4 rows concurrently via col-group tile_position ----
            for w4 in range(GRP // 4):
                wave = g * (GRP // 4) + w4
                cps_all = h_ps.tile([128, H], f32, tag="hx")
                for tc in range(NTC):
                    for r4 in range(4):
                        rr = w4 * 4 + r4
                        nc.tensor.matmul(
                            out=cps_all[32 * r4:32 * r4 + 1, :],
                            lhsT=ats[:, tc * GRP + rr: tc * GRP + rr + 1],
                            rhs=x_rows[rr][:, tc * H:(tc + 1) * H],
                            start=(tc == 0), stop=(tc == NTC - 1),
                            tile_position=(0, 32 * r4),
                        )
                nc.vector.tensor_copy(ctx_sb[:, wave * H:(wave + 1) * H], cps_all[:])

        nc.sync.dma_start(
            out=ctx_d[:, :].rearrange("(w r4) h -> r4 w h", r4=4),
            in_=ctx_sb[:].rearrange("(r4 z) (w h) -> r4 z w h", r4=4, w=RPC // 4)[:, 0, :, :],
        )

    nc.compile()
    return nc


def _get_runner():
    if "runner" in _cached:
        return _cached["runner"]
    sys.path.insert(0, "/opt/trn_rl_repo")
    nc = _build_nc()
    _cached["runner"] = nc
    return nc


def _make_in_maps(inputs):
    import ml_dtypes
    lstm_output = np.ascontiguousarray(inputs["lstm_output"], dtype=np.float32)
    W1, b1, W2 = inputs["W1"], inputs["b1"], inputs["W2"]
    lstm_bf = lstm_output.astype(ml_dtypes.bfloat16)
    # x tiled: [core, row, p, (tc h)] where t = tc*128 + p
    x_tiled = np.ascontiguousarray(
        lstm_bf.reshape(NCORES, RPC, NTC, 128, H).transpose(0, 1, 3, 2, 4)
    ).reshape(NCORES, RPC, 128, NTC * H)
    xt_all = np.ascontiguousarray(
        lstm_bf.reshape(NCORES, RPC, T, H).transpose(0, 1, 3, 2)
    )
    w1_bf = np.ascontiguousarray(W1, dtype=np.float32).astype(ml_dtypes.bfloat16)
    w2_bf = np.concatenate([
        np.ascontiguousarray(W2, dtype=np.float32).reshape(KH, 1),
        np.zeros((KH, 1), np.float32)], axis=1).astype(ml_dtypes.bfloat16)
    eye = np.eye(128, dtype=np.float32)
    in_maps = []
    for c in range(NCORES):
        in_maps.append({
            "x": x_tiled[c],
            "xt": xt_all[c],
            "w1": w1_bf,
            "b1": np.ascontiguousarray(b1, dtype=np.float32).reshape(KH, 1),
            "w2": w2_bf,
            "eye": eye,
        })
    return in_maps


def kernel(lstm_output, W1, b1, W2, b2):
    from concourse.bass_utils import run_bass_kernel_spmd

    nc = _get_runner()
    in_maps = _make_in_maps(dict(lstm_output=lstm_output, W1=W1, b1=b1, W2=W2, b2=b2))
    res = run_bass_kernel_spmd(
        nc, in_maps, core_ids=list(range(NCORES)),
        trace=bool(int(os.environ.get("KERNEL_TRACE", "0"))),
    )
    _cached["last_result"] = res
    outs = res.results
    context = np.concatenate([outs[c]["ctx_out"] / outs[c]["esum_out"] for c in range(NCORES)], axis=0)
    attn = np.concatenate([outs[c]["attn_out"] for c in range(NCORES)], axis=0)
    return context.astype(np.float32), attn.astype(np.float32)


# revision 30
# speedup vs baseline: 1.1149x; 1.0030x over previous
"""Attention-pooling kernel for Trainium2 (8 NeuronCores, pure data parallel).

reference:
    h = tanh(x @ W1 + b1)            # (B,T,128)
    s = (h @ W2 + b2).squeeze(-1)    # (B,T)
    a = softmax(s, axis=1)           # (B,T)   [b2 shifts all scores equally -> no-op]
    c = einsum('bt,bth->bh', a, x)   # (B,H)
    return (c, a)

Sharding: batch dim B=128 split across 8 cores (16 rows each), params replicated.
"""

import os
import sys
import numpy as np

B, T, H, KH = 128, 2048, 256, 128
NCORES = 8
RPC = B // NCORES          # 16 rows per core
GRP = int(os.environ.get('KERNEL_GRP', '8'))  # rows per softmax/pooling group
NTC = T // 128             # 16 t-chunks of 128 per row
NTG = 4                    # t-groups per row (each 4 chunks = 512 t)

_cached = {}


def _build_nc():
    import concourse.bacc as bacc
    import concourse.bass as bass
    import concourse.mybir as mybir
    import concourse.tile as tile
    from contextlib import ExitStack

    f32 = mybir.dt.float32
    MDT = f32 if os.environ.get("KERNEL_MM_DT") == "f32" else mybir.dt.float32r
    AF = mybir.ActivationFunctionType

    nc = bacc.Bacc(None, target_bir_lowering=False)

    bf16 = mybir.dt.bfloat16
    x_d = nc.declare_dram_parameter("x", [RPC, 128, NTC * H], bf16, isOutput=False)
    xt_d = nc.declare_dram_parameter("xt", [RPC, H, T], bf16, isOutput=False)
    w1_d = nc.declare_dram_parameter("w1", [H, KH], bf16, isOutput=False)
    b1_d = nc.declare_dram_parameter("b1", [KH, 1], f32, isOutput=False)
    w2_d = nc.declare_dram_parameter("w2", [KH, 2], bf16, isOutput=False)
    eye_d = nc.declare_dram_parameter("eye", [128, 128], MDT, isOutput=False)
    attn_d = nc.declare_dram_parameter("attn_out", [RPC, T], MDT, isOutput=True)
    ctx_d = nc.declare_dram_parameter("ctx_out", [RPC, H], f32, isOutput=True)
    esum_d = nc.declare_dram_parameter("esum_out", [RPC, 1], f32, isOutput=True)
    DEBUG = bool(int(os.environ.get("KERNEL_DEBUG", "0")))
    if DEBUG:
        dbg_th = nc.declare_dram_parameter("dbg_th", [128, 512], bf16, isOutput=True)
        dbg_ssb = nc.declare_dram_parameter("dbg_ssb", [128, 64], MDT, isOutput=True)
        dbg_rows = nc.declare_dram_parameter("dbg_rows", [GRP, T], f32, isOutput=True)

    with tile.TileContext(nc) as tc, ExitStack() as ctx:
        const = ctx.enter_context(tc.tile_pool(name="const", bufs=1))
        eye_sb = const.tile([128, 128], MDT, tag="eye")
        nc.sync.dma_start(out=eye_sb[:], in_=eye_d[:, :])
        w1_sb = []
        for hc in range(2):
            t_ = const.tile([128, KH], bf16, tag=f"w1_{hc}")
            nc.sync.dma_start(out=t_[:], in_=w1_d[hc * 128:(hc + 1) * 128, :])
            w1_sb.append(t_)
        b1_sb = const.tile([KH, 1], f32, tag="b1")
        nc.sync.dma_start(out=b1_sb[:], in_=b1_d[:, :])
        w2_sb = const.tile([KH, 2], bf16, tag="w2")
        nc.sync.dma_start(out=w2_sb[:], in_=w2_d[:, :])

        x_pool = ctx.enter_context(tc.tile_pool(name="x", bufs=12))
        xt_pool = ctx.enter_context(tc.tile_pool(name="xt", bufs=10))
        xt0_pool = ctx.enter_context(tc.tile_pool(name="xt0", bufs=4))
        h_ps = ctx.enter_context(tc.tile_pool(name="h_ps", bufs=4, space="PSUM"))
        s_ps = ctx.enter_context(tc.tile_pool(name="s_ps", bufs=1, space="PSUM"))
        misc_ps = ctx.enter_context(tc.tile_pool(name="misc_ps", bufs=2, space="PSUM"))
        tanh_p = ctx.enter_context(tc.tile_pool(name="tanh", bufs=4))
        ctx_sb = const.tile([128, (RPC // 4) * H], f32, tag="ctxsb")
        small = ctx.enter_context(tc.tile_pool(name="small", bufs=2))
        rows_p = ctx.enter_context(tc.tile_pool(name="rows", bufs=2))
        dram_p = ctx.enter_context(tc.tile_pool(name="dram", bufs=2, space="DRAM"))

        # HAM warmup: ~5us of back-to-back matmuls while the first DMAs land,
        # so the PE clock-gate opens before real work starts.
        warm_ps = h_ps.tile([128, KH], f32, tag="hx")
        for _ in range(40):
            nc.tensor.matmul(out=warm_ps[:], lhsT=w1_sb[0][:], rhs=w1_sb[1][:],
                             start=True, stop=True)

        for g in range(RPC // GRP):
            x_rows = []
            s_sb_g = small.tile([128, GRP * NTC], MDT, tag="ssb")
            for r4 in range(GRP):
                r = g * GRP + r4
                # xt first (gates the score matmuls); x is only needed at pooling
                if r == 0:
                    # fine-grained first-row xt so the PE starts ~4x earlier
                    xth = []
                    for hc in range(2):
                        row = []
                        for hf in range(2):
                            t4 = xt0_pool.tile([128, T // 2], bf16, tag="xt0")
                            nc.sync.dma_start(
                                out=t4[:],
                                in_=xt_d[r, hc * 128:(hc + 1) * 128,
                                         hf * (T // 2):(hf + 1) * (T // 2)])
                            row.append(t4)
                        xth.append(row)
                    xts = None
                else:
                    xts = []
                    for hc in range(2):
                        xtile = xt_pool.tile([128, T], bf16, tag="xt")
                        nc.sync.dma_start(out=xtile[:], in_=xt_d[r, hc * 128:(hc + 1) * 128, :])
                        xts.append(xtile)
                x_sb = x_pool.tile([128, NTC * H], bf16, tag="x")
                nc.sync.dma_start(out=x_sb[:], in_=x_d[r, :, :])
                x_rows.append(x_sb)

                s_row = s_ps.tile([128, 2 * NTC], f32, tag="s")
                for tg in range(NTG):
                    hps = h_ps.tile([128, 512], f32, tag="hx")
                    for hc in range(2):
                        if r == 0:
                            rhs_ap = xth[hc][tg // 2][:, (tg % 2) * 512:(tg % 2) * 512 + 512]
                        else:
                            rhs_ap = xts[hc][:, tg * 512:(tg + 1) * 512]
                        nc.tensor.matmul(
                            out=hps[:],
                            lhsT=w1_sb[hc][:],
                            rhs=rhs_ap,
                            start=(hc == 0),
                            stop=(hc == 1),
                        )
                    th = tanh_p.tile([128, 512], bf16, tag="tanh")
                    nc.scalar.activation(out=th[:], in_=hps[:], func=AF.Tanh, bias=b1_sb[:])
                    if DEBUG and g == 0 and r4 == 0 and tg == 0:
                        nc.sync.dma_start(out=dbg_th[:, :], in_=th[:])
                    for tc4 in range(4):
                        j = tg * 4 + tc4
                        nc.tensor.matmul(
                            out=s_row[:, 2 * j:2 * j + 2],
                            lhsT=th[:, tc4 * 128:(tc4 + 1) * 128],
                            rhs=w2_sb[:],
                            start=True, stop=True,
                        )
                nc.vector.tensor_copy(
                    s_sb_g[:, r4 * NTC:(r4 + 1) * NTC].rearrange("p j -> p j ()"),
                    s_row[:].rearrange("p (j two) -> p j two", two=2)[:, :, 0:1],
                )

            # ---- per-group: assemble [GRP, T] scores, softmax, attn out ----
            stp = misc_ps.tile([GRP * NTC, 128], MDT, tag="misc")
            nc.tensor.transpose(out=stp[:], in_=s_sb_g[:], identity=eye_sb[:])
            sts = small.tile([GRP * NTC, 128], f32, tag="sts")
            nc.vector.tensor_copy(sts[:], stp[:])
            sts_dram = dram_p.tile([GRP * NTC, 128], f32, tag="sdram")
            nc.sync.dma_start(out=sts_dram[:], in_=sts[:])
            rows = rows_p.tile([GRP, T], f32, tag="rows")
            nc.sync.dma_start(
                out=rows[:].rearrange("r (tc p) -> r tc p", tc=NTC),
                in_=sts_dram[:, :].rearrange("(r tc) p -> r tc p", r=GRP),
            )
            esum = small.tile([GRP, 1], f32, tag="esum")
            rsum = small.tile([GRP, 1], f32, tag="rsum")
            e_sb = rows_p.tile([GRP, T], MDT, tag="e")
            nc.scalar.activation(out=e_sb[:], in_=rows[:], func=AF.Exp, accum_out=esum[:])
            nc.sync.dma_start(out=esum_d[g * GRP:(g + 1) * GRP, :], in_=esum[:])

            # ---- transpose unnormalized exp to [t, r] columns (pooling weights) ----
            atp = misc_ps.tile([128, GRP * NTC], MDT, tag="misc")
            for tc in range(NTC):
                nc.tensor.transpose(
                    out=atp[:, tc * GRP:(tc + 1) * GRP],
                    in_=e_sb[:, tc * 128:(tc + 1) * 128],
                    identity=eye_sb[0:GRP, 0:GRP],
                )
            # normalized attention output (off the pooling critical path)
            attn = rows_p.tile([GRP, T], MDT, tag="attn")
            nc.vector.reciprocal(rsum[:], esum[:])
            nc.vector.tensor_scalar_mul(attn[:], e_sb[:], rsum[:])
            nc.sync.dma_start(out=attn_d[g * GRP:(g + 1) * GRP, :], in_=attn[:])
            ats = small.tile([128, GRP * NTC], bf16, tag="ats")
            nc.vector.tensor_copy(ats[:], atp[:])

            # ---- pooling: 4 rows concurrently via col-group tile_position ----
            for w4 in range(GRP // 4):
                wave = g * (GRP // 4) + w4
                cps_all = h_ps.tile([128, H], f32, tag="hx")
                for tc in range(NTC):
                    for r4 in range(4):
                        rr = w4 * 4 + r4
                        nc.tensor.matmul(
                            out=cps_all[32 * r4:32 * r4 + 1, :],
                            lhsT=ats[:, tc * GRP + rr: tc * GRP + rr + 1],
                            rhs=x_rows[rr][:, tc * H:(tc + 1) * H],
                            start=(tc == 0), stop=(tc == NTC - 1),
                            tile_position=(0, 32 * r4),
                        )
                nc.vector.tensor_copy(ctx_sb[:, wave * H:(wave + 1) * H], cps_all[:])

        nc.sync.dma_start(
            out=ctx_d[:, :].rearrange("(w r4) h -> r4 w h", r4=4),
            in_=ctx_sb[:].rearrange("(r4 z) (w h) -> r4 z w h", r4=4, w=RPC // 4)[:, 0, :, :],
        )

    nc.compile()
    return nc


def _get_runner():
    if "runner" in _cached:
        return _cached["runner"]
    sys.path.insert(0, "/opt/trn_rl_repo")
    nc = _build_nc()
    _cached["runner"] = nc
    return nc


def _make_in_maps(inputs):
    import ml_dtypes
    lstm_output = np.ascontiguousarray(inputs["lstm_output"], dtype=np.float32)
    W1, b1, W2 = inputs["W1"], inputs["b1"], inputs["W2"]
    lstm_bf = lstm_output.astype(ml_dtypes.bfloat16)
    # x tiled: [core, row, p, (tc h)] where t = tc*128 + p
    x_tiled = np.ascontiguousarray(
        lstm_bf.reshape(NCORES, RPC, NTC, 128, H).transpose(0, 1, 3, 2, 4)
    ).reshape(NCORES, RPC, 128, NTC * H)
    xt_all = np.ascontiguousarray(
        lstm_bf.reshape(NCORES, RPC, T, H).transpose(0, 1, 3, 2)
    )
    w1_bf = np.ascontiguousarray(W1, dtype=np.float32).astype(ml_dtypes.bfloat16)
    w2_bf = np.concatenate([
        np.ascontiguousarray(W2, dtype=np.float32).reshape(KH, 1),
        np.zeros((KH, 1), np.float32)], axis=1).astype(ml_dtypes.bfloat16)
    eye = np.eye(128, dtype=np.float32)
    in_maps = []
    for c in range(NCORES):
        in_maps.append({
            "x": x_tiled[c],
            "xt": xt_all[c],
            "w1": w1_bf,
            "b1": np.ascontiguousarray(b1, dtype=np.float32).reshape(KH, 1),
            "w2": w2_bf,
            "eye": eye,
        })
    return in_maps


def kernel(lstm_output, W1, b1, W2, b2):
    from concourse.bass_utils import run_bass_kernel_spmd

    nc = _get_runner()
    in_maps = _make_in_maps(dict(lstm_output=lstm_output, W1=W1, b1=b1, W2=W2, b2=b2))
    res = run_bass_kernel_spmd(
        nc, in_maps, core_ids=list(range(NCORES)),
        trace=bool(int(os.environ.get("KERNEL_TRACE", "0"))),
    )
    _cached["last_result"] = res
    outs = res.results
    context = np.concatenate([outs[c]["ctx_out"] / outs[c]["esum_out"] for c in range(NCORES)], axis=0)
    attn = np.concatenate([outs[c]["attn_out"] for c in range(NCORES)], axis=0)
    return context.astype(np.float32), attn.astype(np.float32)
